# revision 5
# baseline (speedup 1.0000x reference)
"""Child-Sum Tree-LSTM cell on 8 Trainium2 NeuronCores (Bass/Tile).

Data-parallel over the batch axis: each core gets B/8 = 4096 rows of
x/h/C plus replicated [128,128] weights, computes (h_j, c_j) for its
shard, and the host concatenates the shards.

Per-core kernel (per 128-row tile):
  - DMA-cast x/h/C fp32->fp16 into SBUF (SWDGE casts inline).
  - PE transposes x and each h_n (fp16, via identity) so they can serve
    as matmul stationaries; h_tilde^T = sum_n h_n^T accumulated in PSUM
    by 8 plain matmuls against the identity.
  - Gate pre-activations assembled entirely in PSUM accumulation:
      A[:, 0:384]  = x@[Wi|Wo|Wu] + 1(x)[bi|bo|bu] + h_tilde@[Ui|Uo|Uu]
      F[:, n*128:] = x@Wf + 1(x)bf + h_n@Uf      (all 8 children)
    (biases enter as rank-1 K=1 matmuls with a ones stationary)
  - ScalarE applies sigmoid/tanh straight out of PSUM (fp16 out).
  - VectorE does the f (.) C multiply + child-tree reduction and the
    final c = i*u + fc, h = o*tanh(c) (fp32 outputs).
"""

import numpy as np

D = 128
NCH = 8
NCORES = 8
BATCH = 32768
P = 128

_CACHE = {}


def build_nc(b_loc, variant="full"):
    from contextlib import ExitStack

    import concourse.tile as tile
    from concourse import bacc, mybir
    from concourse.masks import make_identity

    f32 = mybir.dt.float32
    f16 = mybir.dt.float16

    ntiles = b_loc // P
    assert b_loc % P == 0

    nc = bacc.Bacc("TRN2", target_bir_lowering=False, debug=False)

    x_d = nc.dram_tensor("x", [b_loc, D], f32, kind="ExternalInput")
    h_d = nc.dram_tensor("h", [NCH, b_loc, D], f32, kind="ExternalInput")
    C_d = nc.dram_tensor("C", [NCH, b_loc, D], f32, kind="ExternalInput")
    Wd = {
        n: nc.dram_tensor(n, [D, D], f32, kind="ExternalInput")
        for n in ("W_i", "W_f", "W_o", "W_u", "U_i", "U_f", "U_o", "U_u")
    }
    bd = {
        n: nc.dram_tensor(n, [1, D], f32, kind="ExternalInput")
        for n in ("b_i", "b_f", "b_o", "b_u")
    }
    h_o = nc.dram_tensor("h_out", [b_loc, D], f32, kind="ExternalOutput")
    c_o = nc.dram_tensor("c_out", [b_loc, D], f32, kind="ExternalOutput")

    with ExitStack() as ctx:
        tc = ctx.enter_context(tile.TileContext(nc))
        consts = ctx.enter_context(tc.tile_pool(name="consts", bufs=1))
        loads = ctx.enter_context(tc.tile_pool(name="loads", bufs=3))
        work = ctx.enter_context(tc.tile_pool(name="work", bufs=3))
        outp = ctx.enter_context(tc.tile_pool(name="outp", bufs=3))
        tp_ps = ctx.enter_context(tc.tile_pool(name="tp_ps", bufs=1, space="PSUM"))
        hs_ps = ctx.enter_context(tc.tile_pool(name="hs_ps", bufs=1, space="PSUM"))
        a_ps = ctx.enter_context(tc.tile_pool(name="a_ps", bufs=2, space="PSUM"))
        f_ps = ctx.enter_context(tc.tile_pool(name="f_ps", bufs=1, space="PSUM"))

        # ---- one-time constants -------------------------------------------
        ident = consts.tile([P, P], f16)
        make_identity(nc, ident)
        ones = consts.tile([1, P], f16)
        nc.vector.memset(ones, 1.0)

        Wcat = consts.tile([P, 3, D], f16)  # [Wi|Wo|Wu]
        Ucat = consts.tile([P, 3, D], f16)  # [Ui|Uo|Uu]
        bcat = consts.tile([1, 3, D], f16)  # [bi|bo|bu]
        for j, (w, u, b) in enumerate(
            (("W_i", "U_i", "b_i"), ("W_o", "U_o", "b_o"), ("W_u", "U_u", "b_u"))
        ):
            nc.gpsimd.dma_start(Wcat[:, j, :], Wd[w][:, :])
            nc.gpsimd.dma_start(Ucat[:, j, :], Wd[u][:, :])
            nc.gpsimd.dma_start(bcat[:, j, :], bd[b][:, :])
        Uf = consts.tile([P, D], f16)
        nc.gpsimd.dma_start(Uf, Wd["U_f"][:, :])
        Wf4 = consts.tile([P, 4, D], f16)  # W_f replicated 4x (one PSUM bank wide)
        bf4 = consts.tile([1, 4, D], f16)
        for j in range(4):
            nc.gpsimd.dma_start(Wf4[:, j, :], Wd["W_f"][:, :])
            nc.gpsimd.dma_start(bf4[:, j, :], bd["b_f"][:, :])

        if variant == "dma_only":
            zc = consts.tile([P, D], f32)
            nc.vector.memset(zc, 0.0)
            zh = consts.tile([P, D], f32)
            nc.vector.memset(zh, 0.0)

        if variant == "compute_only":
            x_sb0 = consts.tile([P, D], f16)
            nc.gpsimd.dma_start(x_sb0, x_d[0:P, :])
            h_sb0 = consts.tile([P, NCH, D], f16)
            nc.gpsimd.dma_start(h_sb0, h_d[:, 0:P, :].rearrange("n b k -> b n k"))
            C_sb0 = consts.tile([P, NCH, D], f16)
            nc.gpsimd.dma_start(C_sb0, C_d[:, 0:P, :].rearrange("n b k -> b n k"))

        reps = 1
        if variant.startswith("rep"):
            reps = int(variant[3:])
            variant = "full"

        # ---- main loop over 128-row tiles ---------------------------------
        for t in range(ntiles * reps):
            t = t % ntiles
            r0 = t * P

            if variant == "compute_only":
                x_sb, h_sb, C_sb = x_sb0, h_sb0, C_sb0
            else:
                x_sb = loads.tile([P, D], f16, tag="x_sb")
                nc.gpsimd.dma_start(x_sb, x_d[r0 : r0 + P, :])
                h_sb = loads.tile([P, NCH, D], f16, tag="h_sb")
                nc.gpsimd.dma_start(
                    h_sb, h_d[:, r0 : r0 + P, :].rearrange("n b k -> b n k")
                )
                C_sb = loads.tile([P, NCH, D], f16, tag="C_sb")
                nc.gpsimd.dma_start(
                    C_sb, C_d[:, r0 : r0 + P, :].rearrange("n b k -> b n k")
                )

            if variant == "dma_only":
                # touch the loaded tiles so DCE keeps the DMAs
                dmy = work.tile([P, 1], f32, tag="dmy")
                nc.vector.tensor_add(dmy, h_sb[:, 0, 0:1], C_sb[:, 0, 0:1])
                nc.vector.tensor_add(dmy, dmy, x_sb[:, 0:1])
                nc.sync.dma_start(c_o[r0 : r0 + P, :], zc)
                nc.sync.dma_start(h_o[r0 : r0 + P, :], zh)
                continue

            # Transposes: h_n^T fill bank0 of tp (8 x 256B), x^T in bank1.
            tp = tp_ps.tile([P, 9, D], f16, tag="tp")
            for n in range(NCH):
                nc.tensor.matmul(
                    tp[:, n, :],
                    h_sb[:, n, :],
                    ident,
                    is_transpose=True,
                    start=(n == 0),
                    stop=(n == NCH - 1),
                )
            nc.tensor.matmul(
                tp[:, 8, :], x_sb, ident, is_transpose=True, start=True, stop=True
            )
            tps = work.tile([P, 9, D], f16, tag="tps")
            nc.scalar.copy(tps, tp)
            xT = tps[:, 8, :]

            # h_tilde^T accumulated in PSUM: sum_n (h_n^T @ I) as plain matmuls.
            hs = hs_ps.tile([P, D], f32, tag="hs")
            for n in range(NCH):
                nc.tensor.matmul(
                    hs, h_sb[:, n, :], ident, start=(n == 0), stop=(n == NCH - 1)
                )
            hsT = work.tile([P, D], f16, tag="hsT")
            nc.scalar.copy(hsT, hs)

            # A = x@[Wi|Wo|Wu] + 1(x)[bi|bo|bu] + h_tilde@[Ui|Uo|Uu]
            A = a_ps.tile([P, 3, D], f32, tag="A")
            nc.tensor.matmul(A, xT, Wcat, start=True, stop=False)
            nc.tensor.matmul(A, ones, bcat, start=False, stop=False)
            nc.tensor.matmul(A, hsT, Ucat, start=False, stop=True)

            # F_n = x@Wf + 1(x)bf + h_n@Uf, children 0..3 in bank0, 4..7 bank1.
            F = f_ps.tile([P, NCH, D], f32, tag="F")
            for j in range(2):
                blk = F[:, 4 * j : 4 * j + 4, :]
                nc.tensor.matmul(blk, xT, Wf4, start=True, stop=False)
                nc.tensor.matmul(blk, ones, bf4, start=False, stop=False)
                for c in range(4):
                    n = 4 * j + c
                    nc.tensor.matmul(
                        F[:, n, :],
                        tps[:, n, :],
                        Uf,
                        start=False,
                        stop=(c == 3),
                    )

            # Gates straight out of PSUM on ScalarE.
            Sig = mybir.ActivationFunctionType.Sigmoid
            Tanh = mybir.ActivationFunctionType.Tanh
            io_sb = work.tile([P, 2, D], f16, tag="io_sb")
            nc.scalar.activation(io_sb, A[:, 0:2, :], Sig)
            u_sb = work.tile([P, D], f16, tag="u_sb")
            nc.scalar.activation(u_sb, A[:, 2, :], Tanh)
            f_sb = work.tile([P, NCH, D], f16, tag="f_sb")
            nc.scalar.activation(f_sb, F, Sig)

            # fc = sum_n f_n * C_n (fp16 tree on VectorE, 2x mode).
            prod = work.tile([P, NCH, D], f16, tag="prod")
            nc.vector.tensor_mul(prod, f_sb, C_sb)
            s1 = work.tile([P, 4, D], f16, tag="s1")
            nc.vector.tensor_add(s1, prod[:, 0:4, :], prod[:, 4:8, :])
            s2 = work.tile([P, 2, D], f16, tag="s2")
            nc.vector.tensor_add(s2, s1[:, 0:2, :], s1[:, 2:4, :])
            fc = work.tile([P, D], f16, tag="fc")
            nc.vector.tensor_add(fc, s2[:, 0, :], s2[:, 1, :])

            # c = i*u + fc ; h = o*tanh(c)  (fp32 outputs)
            iu = work.tile([P, D], f16, tag="iu")
            nc.vector.tensor_mul(iu, io_sb[:, 0, :], u_sb)
            c_sb = outp.tile([P, D], f32, tag="c_sb")
            nc.vector.tensor_add(c_sb, iu, fc)
            t_sb = work.tile([P, D], f16, tag="t_sb")
            nc.scalar.activation(t_sb, c_sb, Tanh)
            hh_sb = outp.tile([P, D], f32, tag="hh_sb")
            nc.vector.tensor_mul(hh_sb, io_sb[:, 1, :], t_sb)

            nc.sync.dma_start(c_o[r0 : r0 + P, :], c_sb)
            nc.sync.dma_start(h_o[r0 : r0 + P, :], hh_sb)

    nc.compile()
    return nc


def _shard_inputs(inputs, b_loc):
    n_shards = inputs["x"].shape[0] // b_loc
    in_maps = []
    for i in range(n_shards):
        s = slice(i * b_loc, (i + 1) * b_loc)
        m = {}
        for k, v in inputs.items():
            v = np.ascontiguousarray(np.asarray(v), dtype=np.float32)
            if k == "x":
                m[k] = np.ascontiguousarray(v[s])
            elif k in ("h", "C"):
                m[k] = np.ascontiguousarray(v[:, s])
            else:
                m[k] = v
        in_maps.append(m)
    return in_maps


def kernel(**inputs):
    from concourse.bass_utils import run_bass_kernel_spmd

    b_loc = BATCH // NCORES
    if b_loc not in _CACHE:
        _CACHE[b_loc] = build_nc(b_loc)
    nc = _CACHE[b_loc]

    in_maps = _shard_inputs(inputs, b_loc)
    res = run_bass_kernel_spmd(nc, in_maps, core_ids=list(range(NCORES)))
    h_full = np.concatenate([r["h_out"] for r in res.results], axis=0)
    c_full = np.concatenate([r["c_out"] for r in res.results], axis=0)
    return (h_full, c_full)
